# revision 17
# baseline (speedup 1.0000x reference)
"""Trainium2 Bass kernel for the supervoxel erode/edge loss module.

The reference divides a padded [B,X,Y] grid (pad offset 4*sx along x, 4*sy
along y) into 8x8 patches, zeroes the last row/col of the mask channel in
each patch, erodes along both patch axes and sums eroded*edge. The erode
`a*b + (1-a)*a + (1-b)*a` equals `2a - a^2` with a = m(i)*m(i+1), and the
whole module collapses to a global elementwise expression on the grid:

    mt(x,y) = mask[b,x,y,idx] * [(x+4sx)%8 != 7] * [(y+4sy)%8 != 7]
    ax = mt(x,y)*mt(x+1,y); ay = mt(x,y)*mt(x,y+1)   (zero past image edge)
    total = sum_b,x,y ax(2-ax) * ay(2-ay) * edge
    out = loss_old + total / (B * ((X+8)//8) * ((Y+8)//8))

Performance structure (informed by NTFF traces):
  * DMA is the roofline. Each SDMA engine moves ~13 GB/s; only SWDGE
    (gpsimd-issued) mask DMAs with one 16 KiB descriptor per (partition,
    tile) spread across all 16 engines (~208 GB/s). HWDGE trigger
    instructions also cost ~0.1 us per descriptor on the issuing engine,
    so the edge uses few large HWDGE DMAs on the otherwise idle Sync
    engine. GpSimd runs no compute so SWDGE descriptor generation is
    never blocked.
  * Tiles: 120 grid rows per tile, one row per partition (121st partition
    overlaps the next tile for the x+1 neighbor); 120%8==0 makes the row
    mask R per-partition and tile-invariant (rvec applied once at the end).
  * Compute is dense bf16: ACT extracts the mask channel (strided f32 read
    with cast) and computes P=(1-ax)^2, Q=(1-ay)^2; PE shifts rows via a
    matmul with a bf16 shift matrix; DVE does 2 muls, one stt, and a final
    stt with fused per-partition accumulation over the live columns only
    (col mask C folds into the strided view: nx*ny = (P-1)*(Q-1), so
    contribution = (P-1) * [(Q-1)*edge]).

Sharding: data-parallel over batch, B/8 images per core; per-core partial
sums are combined on the host (final result is one scalar).
"""

import sys

sys.path.insert(0, "/opt/trn_rl_repo")

import numpy as np

from concourse import bacc, mybir, tile
from concourse.ap import AP
from concourse.bass_utils import run_bass_kernel_spmd

F32 = mybir.dt.float32
BF16 = mybir.dt.bfloat16
N_CORES = 8
TS = 120  # grid rows per tile (multiple of 8 so p%8 row pattern is invariant)
SHIFTS = [(0, 0), (1, 0), (0, 1), (1, 1)]


def _col_runs(sy: int, Y: int):
    """Live contribution columns as strided runs (off, ngroups, runlen):
    cols y = off + 8*g + k, k in [0, runlen). A contribution col y is live
    iff (y+4sy)%8 not in {6,7}. Runs stay within [0, Y-2]."""
    if sy == 0:
        return [(0, Y // 8, 6)]
    # sy == 1: dead {2,3}; live = {4..9} wrapping: main run + head + tail
    return [(4, Y // 8 - 1, 6), (0, 1, 2), (Y - 4, 1, 3)]


def _build_program(Bc: int, X: int, Y: int, idx: int):
    """Per-core program. Inputs: mask [Bc,X,Y,4] f32, edge [Bc,X,Y,1] f32,
    smat [128,128] bf16, rvec [128,1] f32, ones [128,1] f32.
    Output: out [1,1] f32 partial sum (row/col masked, unscaled)."""
    sx, sy = SHIFTS[idx]
    assert X % 8 == 0 and Y % 8 == 0
    nk = (X + TS - 1) // TS  # tiles per image
    npair = nk // 2
    odd_rows = X - 2 * TS * npair  # rows of trailing unpaired tile
    runs = _col_runs(sy, Y)
    nruns = len(runs)
    NT = Bc * nk * nruns
    row = Y * 4  # f32 elements per grid row (4 channels)
    W = Y - 1

    nc = bacc.Bacc("TRN2", target_bir_lowering=False, debug=False)
    mask_h = nc.dram_tensor("mask", [Bc, X, Y, 4], F32, kind="ExternalInput")
    edge_h = nc.dram_tensor("edge", [Bc, X, Y, 1], F32, kind="ExternalInput")
    smat_h = nc.dram_tensor("smat", [128, 128], BF16, kind="ExternalInput")
    rvec_h = nc.dram_tensor("rvec", [128, 1], F32, kind="ExternalInput")
    ones_h = nc.dram_tensor("ones", [128, 1], F32, kind="ExternalInput")
    out_h = nc.dram_tensor("out", [1, 1], F32, kind="ExternalOutput")

    def mask_pair_src(b, m):
        """Overlapping-window DRAM AP [TS+1, 2, Y*4]: (p, j) reads the 16 KiB
        row 2*TS*m + TS*j + p of image b (j = tile within the pair)."""
        off = (b * X + 2 * TS * m) * row
        return AP(mask_h, off, [[row, TS + 1], [TS * row, 2], [1, row]])

    with tile.TileContext(nc) as tc:
        with (
            tc.tile_pool(name="mt", bufs=3) as mt_pool,
            tc.tile_pool(name="et", bufs=3) as et_pool,
            tc.tile_pool(name="eto", bufs=2) as eto_pool,
            tc.tile_pool(name="vd", bufs=3) as vd_pool,
            tc.tile_pool(name="shc", bufs=2) as shc_pool,
            tc.tile_pool(name="ax", bufs=2) as ax_pool,
            tc.tile_pool(name="ay", bufs=2) as ay_pool,
            tc.tile_pool(name="pq", bufs=2) as pq_pool,
            tc.tile_pool(name="ff", bufs=2) as f_pool,
            tc.tile_pool(name="psum", bufs=2, space="PSUM") as ps_pool,
            tc.tile_pool(name="psum1", bufs=1, space="PSUM") as ps1_pool,
            tc.tile_pool(name="const", bufs=1) as c_pool,
        ):
            smat_t = c_pool.tile([128, 128], BF16)
            rvec_t = c_pool.tile([128, 1], F32)
            ones_t = c_pool.tile([128, 1], F32)
            partials = c_pool.tile([128, NT], F32)
            nc.sync.dma_start(smat_t[:], smat_h.ap())
            nc.sync.dma_start(rvec_t[:], rvec_h.ap())
            nc.sync.dma_start(ones_t[:], ones_h.ap())
            nc.vector.memset(partials[:], 0.0)

            def emit_compute(mv, ev, P2, cr, slot):
                """mv: [P2, Y] stride-4 f32 view of the mask channel;
                ev: [cr(+), Y] f32 edge view; accumulates the tile's
                row/col-masked partial sums into partials[0:cr, slot...]."""
                # channel extract + cast -> dense bf16 (ACT)
                vd = vd_pool.tile([P2, Y], BF16)
                nc.scalar.copy(vd[:], mv)
                # x+1 row shift via PE matmul (bf16 in, f32 PSUM out)
                shp = ps_pool.tile([128, Y], F32)
                for c0 in range(0, Y, 512):
                    nc.tensor.matmul(
                        shp[:, c0 : c0 + 512],
                        smat_t[0:P2, :],
                        vd[:, c0 : c0 + 512],
                        start=True,
                        stop=True,
                    )
                shc = shc_pool.tile([cr, Y], BF16)
                nc.vector.tensor_copy(shc[:], shp[0:cr, :])
                # ax = v * v(x+1);  ay = v * v(y+1)
                ax = ax_pool.tile([cr, Y], BF16)
                nc.vector.tensor_mul(ax[:], vd[0:cr, :], shc[:])
                ay = ay_pool.tile([cr, Y], BF16)
                nc.vector.tensor_mul(ay[:, 0:W], vd[0:cr, 0:W], vd[0:cr, 1:Y])
                # P = (1-ax)^2, Q = (1-ay)^2 (ACT); nx*ny = (P-1)*(Q-1)
                pt = pq_pool.tile([cr, Y], BF16)
                qt = pq_pool.tile([cr, Y], BF16)
                nc.scalar.activation(
                    pt[:],
                    ax[:],
                    mybir.ActivationFunctionType.Square,
                    bias=1.0,
                    scale=-1.0,
                )
                nc.scalar.activation(
                    qt[:, 0:W],
                    ay[:, 0:W],
                    mybir.ActivationFunctionType.Square,
                    bias=1.0,
                    scale=-1.0,
                )
                # F = (Q-1)*edge; then (P-1)*F summed over live cols only
                ft = f_pool.tile([cr, Y], BF16)
                nc.vector.scalar_tensor_tensor(
                    ft[:, 0:W],
                    qt[:, 0:W],
                    1.0,
                    ev[0:cr, 0:W],
                    op0=mybir.AluOpType.subtract,
                    op1=mybir.AluOpType.mult,
                )
                for r, (off, ng, rl) in enumerate(runs):
                    g0 = off // 8
                    k0 = off % 8
                    view = lambda tl: tl.rearrange("p (g k) -> p g k", k=8)[
                        :, g0 : g0 + ng, k0 : k0 + rl
                    ]
                    nc.vector.scalar_tensor_tensor(
                        view(qt),
                        view(pt),
                        1.0,
                        view(ft),
                        op0=mybir.AluOpType.subtract,
                        op1=mybir.AluOpType.mult,
                        accum_out=partials[0:cr, slot + r : slot + r + 1],
                    )

            for b in range(Bc):
                eto = None
                if odd_rows:
                    eto = eto_pool.tile([odd_rows, Y], F32)
                    nc.sync.dma_start(
                        eto[:], edge_h.ap()[b, 2 * TS * npair : X, :, 0]
                    )
                for m in range(npair):
                    # alternate mask pairs between the SWDGE queue (gpsimd)
                    # and a HWDGE ring (sync): each queue's descriptor
                    # emission is rate-limited, two queues together keep all
                    # 16 SDMA engines at their ~13 GB/s cap.
                    meng = nc.gpsimd if m % 2 == 0 else nc.sync
                    mtp = mt_pool.tile([TS + 1, 2, Y, 4], F32)
                    meng.dma_start(
                        mtp.rearrange("p j y c -> p j (y c)"), mask_pair_src(b, m)
                    )
                    # per-pair edge rows (4 KiB descriptors, idle Sync engine)
                    etp = et_pool.tile([TS, 2, Y], F32)
                    nc.sync.dma_start(
                        etp[:],
                        AP(
                            edge_h,
                            (b * X + 2 * TS * m) * Y,
                            [[Y, TS], [TS * Y, 2], [1, Y]],
                        ),
                    )
                    for j in range(2):
                        k = 2 * m + j
                        emit_compute(
                            mtp[:, j, :, idx],
                            etp[:, j, :],
                            TS + 1,
                            TS,
                            (b * nk + k) * nruns,
                        )
                if odd_rows:
                    mto = mt_pool.tile([odd_rows, Y, 4], F32)
                    nc.gpsimd.dma_start(
                        mto.rearrange("p y c -> p (y c)"),
                        AP(
                            mask_h,
                            (b * X + 2 * TS * npair) * row,
                            [[row, odd_rows], [1, row]],
                        ),
                    )
                    emit_compute(
                        mto[:, :, idx],
                        eto[:],
                        odd_rows,
                        odd_rows,
                        (b * nk + nk - 1) * nruns,
                    )

            # total = sum_p rvec[p] * sum_t partials[p, t]
            red = c_pool.tile([128, 1], F32)
            rm = c_pool.tile([128, 1], F32)
            nc.vector.reduce_sum(red[:], partials[:], axis=mybir.AxisListType.X)
            nc.vector.tensor_mul(rm[:], red[:], rvec_t[:])
            out_ps = ps1_pool.tile([1, 1], F32)
            nc.tensor.matmul(out_ps[:], rm[:], ones_t[:], start=True, stop=True)
            out_sb = c_pool.tile([1, 1], F32)
            nc.vector.tensor_copy(out_sb[:], out_ps[:])
            nc.sync.dma_start(out_h.ap(), out_sb[:])

    nc.compile()
    return nc


def _host_consts(idx: int):
    import ml_dtypes

    sx, _ = SHIFTS[idx]
    smat = np.zeros((128, 128), np.float32)
    for p in range(127):
        smat[p + 1, p] = 1.0
    xs = np.arange(128)
    rvec = (
        (((xs + 4 * sx) % 8 != 7) & ((xs + 1 + 4 * sx) % 8 != 7))
        .astype(np.float32)
        .reshape(128, 1)
    )
    return smat.astype(ml_dtypes.bfloat16), rvec, np.ones((128, 1), np.float32)


def _run(mask, edge, loss_old, idx, trace=False, **build_kwargs):
    B, X, Y, _ = mask.shape
    assert B % N_CORES == 0
    Bc = B // N_CORES

    nc = _build_program(Bc, X, Y, idx, **build_kwargs)
    smat, rvec, ones = _host_consts(idx)
    in_maps = [
        {
            "mask": mask[i * Bc : (i + 1) * Bc],
            "edge": edge[i * Bc : (i + 1) * Bc],
            "smat": smat,
            "rvec": rvec,
            "ones": ones,
        }
        for i in range(N_CORES)
    ]
    res = run_bass_kernel_spmd(nc, in_maps, list(range(N_CORES)), trace=trace)
    total = float(sum(float(res.results[i]["out"][0, 0]) for i in range(N_CORES)))
    n_patch = ((X + 8) // 8) * ((Y + 8) // 8)
    out = np.float32(np.asarray(loss_old, dtype=np.float32) + total / (B * n_patch))
    return np.asarray(out, dtype=np.float32), res


def kernel(resized_image, mask_combined, edge_map, loss_old, mask_index):
    mask = np.ascontiguousarray(np.asarray(mask_combined, dtype=np.float32))
    edge = np.ascontiguousarray(np.asarray(edge_map, dtype=np.float32))
    idx = int(np.asarray(mask_index))
    out, _ = _run(mask, edge, loss_old, idx)
    return out
